# revision 1
# baseline (speedup 1.0000x reference)
"""Distributed multi-head attention kernel for one TRN2 chip (8 NeuronCores).

Problem: x[2,2048,1024] -> qkv -> 16-head attention -> out proj, f32 I/O.

Sharding: 8 cores = 2 batches x 4 head-groups (4 heads each).
Core c: batch b=c//4, head group g=c%4 (heads 4g..4g+3).

Structure:
 - ScalarE does ONLY the softmax exps (the irreducible ~16.8M elem/core
   bottleneck, ~147us floor); q/k/v/proj PSUM evictions run on DVE.
   v-production and the pair-1 qkT are emitted before the attention
   combos (Tile deps follow emission order) but priority-DEMOTED so the
   scheduler slots them into PE slack under the exp stream.
 - Queue discipline to avoid head-of-line blocking: o->ag_in stores on
   the SP HWDGE queue; gathered-o loads on gpsimd SWDGE (their wait on
   the AllGather semaphore only delays later collectives, which are
   serial anyway); final out-stores detached to the end of the program.
 - The final (pair1, qtile3) combo is split into two half-width
   sub-combos with two small AllGathers so the last gather+projection
   pipeline under compute (shorter serial tail).
 - A dummy 16B AllGather at T=0 absorbs the ~10-45us collectives
   bootstrap barrier into the input-DMA window; a tiny exp preloads the
   ACT exp table; input DMAs alternate between the SP/ACT HWDGE queues.

Device algorithm (per core), all matmuls bf16 with f32 PSUM accumulate:
 1) qkT = wqk.T @ xT   [512,2048]  (q weights/bias pre-scaled by
    1/sqrt(dh) on host; bias added during DVE eviction)
 2) v_aug = xT.T @ w_v [2048, 4*65] (bias via DVE; per-head ones column
    interleaved so the PV matmul also emits softmax denominators)
 3) per (pair, q-range): per chunk-pair: sT = k @ qT (row-tiled head
    pair), p = exp(sT) on ScalarE (PSUM->SBUF bf16), oT_aug += v_aug.T
    @ p (M=65: row 64 = denominator); normalize oT by 1/denom on DVE.
 4) per q-range: AllGather o of both pairs (256 rows); proj with w_proj
    chunks stationary: out.T[of, q] += wp.T @ o_full; + bias; DMA out
    [256, 2048] transposed (host un-transposes).
"""

import os
import sys
import types
from collections import deque
import numpy as np
import ml_dtypes

import concourse.bass as bass
import concourse.mybir as mybir
import concourse.bacc as bacc
import concourse.tile as tile
from concourse.bass_utils import run_bass_kernel_spmd

BF16 = mybir.dt.bfloat16
F32 = mybir.dt.float32

B, N, D = 2, 2048, 1024
H, DH = 16, 64
SCALE = DH ** -0.5

P = 128                 # partitions
NT = 512                # token free-dim tile
KC = N // P             # 16 k-token chunks
QT = N // NT            # 4 q tiles
DC = D // P             # 8 d_model chunks
HPC = 4                 # heads per core
OF = HPC * DH           # 256 o-features per core
VW = HPC * (DH + 1)     # v_aug width (260): per head [v(64) | ones(1)]

CORE_IDS = list(range(8))
GROUPS = [[0, 1, 2, 3], [4, 5, 6, 7]]
LAST_RESULTS = None


def _install_ntff_shim():
    """Provide antenv.axon_hooks (absent from this image's antenv stub) so
    run_bass_kernel_spmd(trace=True) can reach the NTFF profiler in
    libaxon_pjrt.so. Only needed when profiling."""
    if "antenv.axon_hooks" in sys.modules:
        return
    try:
        from trn_agent_boot.trn_boot import _ntff_profile_via_ctypes
        hook = _ntff_profile_via_ctypes("/opt/axon/libaxon_pjrt.so")
    except Exception:
        hook = None
    mod = types.ModuleType("antenv.axon_hooks")
    mod._hook = hook
    mod.get_axon_ntff_profile_hook = lambda: mod._hook
    mod.set_axon_ntff_profile_hook = lambda h: setattr(mod, "_hook", h)
    sys.modules["antenv.axon_hooks"] = mod


def build_nc():
    nc = bacc.Bacc("TRN2", target_bir_lowering=False, debug=False, num_devices=8)

    xt_ext = nc.dram_tensor("xt", [D, N], BF16, kind="ExternalInput")
    wqk_ext = nc.dram_tensor("wqk", [D, 2 * OF], BF16, kind="ExternalInput")
    wv_ext = nc.dram_tensor("wv", [D, OF], BF16, kind="ExternalInput")
    bqk_ext = nc.dram_tensor("bqk", [P, 4], F32, kind="ExternalInput")
    bv_ext = nc.dram_tensor("bv", [1, OF], F32, kind="ExternalInput")
    wp_ext = nc.dram_tensor("wp", [D, OF], BF16, kind="ExternalInput")
    bp_ext = nc.dram_tensor("bp", [P, 2], F32, kind="ExternalInput")
    # transposed output [of, q]; host transposes back
    out_ext = nc.dram_tensor("out", [OF, N], F32, kind="ExternalOutput")

    # AllGather bounce buffers per q-range (qt 0,1,2 full; qt3 in halves —
    # ranges 3,4 — so the tail gather pipelines; range 5 = full qt3, used
    # only by pair 0's attention which feeds both half buffers).
    # Separate tensors so Tile's whole-tensor DRAM dep tracking never
    # falsely serializes.  Ranges: (q0, qw).
    QRANGES = [(0, NT), (NT, NT), (2 * NT, NT),
               (3 * NT, NT // 2), (3 * NT + NT // 2, NT // 2),
               (3 * NT, NT)]
    ag_in = [nc.dram_tensor(f"ag_in_{i}", [2 * P, qw], BF16)
             for i, (q0, qw) in enumerate(QRANGES[:5])]
    ag_out = [nc.dram_tensor(f"ag_out_{i}", [8 * P, qw], BF16)
              for i, (q0, qw) in enumerate(QRANGES[:5])]
    # dummy collective to absorb the bootstrap barrier at T=0
    agw_in = nc.dram_tensor("agw_in", [1, 16], BF16)
    agw_out = nc.dram_tensor("agw_out", [4, 16], BF16)

    with tile.TileContext(nc) as tc:
        with (
            tc.tile_pool(name="xt_pool", bufs=1) as xt_pool,
            tc.tile_pool(name="w_pool", bufs=1) as w_pool,
            tc.tile_pool(name="qk_pool", bufs=1) as qk_pool,
            tc.tile_pool(name="v_pool", bufs=1) as v_pool,
            tc.tile_pool(name="const_pool", bufs=1) as const_pool,
            tc.tile_pool(name="pt_pool", bufs=6) as pt_pool,
            tc.tile_pool(name="o_pool", bufs=4) as o_pool,
            tc.tile_pool(name="nrm_pool", bufs=2) as nrm_pool,
            tc.tile_pool(name="ofull_pool", bufs=10) as ofull_pool,
            tc.tile_pool(name="pr_pool", bufs=1) as pr_pool,
            tc.tile_pool(name="sw_pool", bufs=2, space="PSUM") as sw_pool,
            tc.tile_pool(name="po_pool", bufs=1, space="PSUM") as po_pool,
            tc.tile_pool(name="aux_pool", bufs=2, space="PSUM") as aux_pool,
        ):
            # ---- T=0 warmups ------------------------------------------------
            nc.gpsimd.collective_compute(
                "AllGather", mybir.AluOpType.bypass,
                replica_groups=GROUPS,
                ins=[agw_in.ap().opt()],
                outs=[agw_out.ap().opt()])
            warm_in = const_pool.tile([1, 16], F32)
            warm_out = const_pool.tile([1, 16], F32)
            nc.vector.memset(warm_in[:], 0.0)
            nc.scalar.activation(
                warm_out[:], warm_in[:], mybir.ActivationFunctionType.Exp)

            # ---- input loads: wqk+xt pairs first on the two HWDGE queues
            #      (they gate phase B), then consts, wv, wp.
            xt_t, wqk_t, wv_t = [], [], []
            for k in range(DC):
                eng = nc.sync if (k % 2 == 0) else nc.scalar
                tw = w_pool.tile([P, 2 * OF], BF16, name=f"wqk{k}")
                eng.dma_start(tw[:], wqk_ext[k * P:(k + 1) * P, :])
                wqk_t.append(tw)
                eng2 = nc.scalar if (k % 2 == 0) else nc.sync
                tx = xt_pool.tile([P, N], BF16, name=f"xt{k}")
                eng2.dma_start(tx[:], xt_ext[k * P:(k + 1) * P, :])
                xt_t.append(tx)

            bqk_sb = const_pool.tile([P, 4], F32)
            nc.sync.dma_start(bqk_sb[:], bqk_ext[:])
            bv_row = const_pool.tile([1, OF], F32)
            nc.sync.dma_start(bv_row[:], bv_ext[:])
            bp_sb = const_pool.tile([P, 2], F32)
            nc.scalar.dma_start(bp_sb[:], bp_ext[:])
            bv_bc = const_pool.tile([P, OF], F32)
            nc.gpsimd.partition_broadcast(bv_bc[:], bv_row[:])

            for k in range(DC):
                eng = nc.sync if (k % 2 == 0) else nc.scalar
                t = w_pool.tile([P, OF], BF16, name=f"wv{k}")
                eng.dma_start(t[:], wv_ext[k * P:(k + 1) * P, :])
                wv_t.append(t)
            wp_t = {}
            for pair in range(2):
                for g in range(4):
                    eng = nc.sync if ((pair + g) % 2 == 0) else nc.scalar
                    t = w_pool.tile([P, OF], BF16, name=f"wp{pair}{g}")
                    r0 = 256 * g + 128 * pair
                    eng.dma_start(t[:], wp_ext[r0:r0 + P, :])
                    wp_t[(pair, g)] = t

            # ---- filler framework ------------------------------------------
            # Units of ~0.5-1.5us of TensorE work, drained (emitted) between
            # attention chunk iterations so the scheduler can slot them into
            # PE slack at fine grain.
            FILL = deque()

            def drain(n):
                for _ in range(n):
                    if FILL:
                        FILL.popleft()()

            # ---- phase B: qkT = wqk.T @ xT -> 4 tiles [128, 2048] bf16 ------
            # m=0: q heads 0-1, m=1: q heads 2-3, m=2: k heads 0-1, m=3: k 2-3
            qk_sb = [qk_pool.tile([P, N], BF16, name=f"qk{m}") for m in range(4)]

            def qk_mtile_units(m):
                """qkT m-tile as 4 units of (np, k-half): LDW-amortized
                n-pair inner loop, DVE eviction with bias on the last."""
                units = []
                for np_ in range(2):
                    state = {}

                    def make(np_, kh, state):
                        def u():
                            if kh == 0:
                                state["ps"] = [
                                    aux_pool.tile([P, NT], F32, name="aux")
                                    for _ in range(2)]
                            for k in range(4 * kh, 4 * kh + 4):
                                for j in range(2):
                                    n = 2 * np_ + j
                                    nc.tensor.matmul(
                                        state["ps"][j][:],
                                        wqk_t[k][:, m * P:(m + 1) * P],
                                        xt_t[k][:, n * NT:(n + 1) * NT],
                                        start=(k == 0), stop=(k == DC - 1))
                            if kh == 1:
                                for j in range(2):
                                    n = 2 * np_ + j
                                    nc.vector.tensor_scalar_add(
                                        qk_sb[m][:, n * NT:(n + 1) * NT],
                                        state["ps"][j][:],
                                        bqk_sb[:, m:m + 1])
                        return u
                    units.append(make(np_, 0, state))
                    units.append(make(np_, 1, state))
                return units

            def qk_mtile(m):
                for u in qk_mtile_units(m):
                    u()

            # ---- phase C: v_aug [2048, 260] bf16 (ones interleaved) ---------
            v_sb = [v_pool.tile([P, VW], BF16, name=f"v{t}") for t in range(KC)]

            def v_units(t):
                state = {}

                def ua():
                    state["ps"] = aux_pool.tile([P, OF], F32, name="aux")
                    for k in range(4):
                        nc.tensor.matmul(
                            state["ps"][:], xt_t[k][:, t * P:(t + 1) * P],
                            wv_t[k][:], start=(k == 0), stop=False)

                def ub():
                    ps = state["ps"]
                    for k in range(4, 8):
                        nc.tensor.matmul(
                            ps[:], xt_t[k][:, t * P:(t + 1) * P], wv_t[k][:],
                            start=False, stop=(k == DC - 1))
                    vdst = v_sb[t][:, :].rearrange("p (h c) -> p h c", c=DH + 1)
                    nc.vector.tensor_add(
                        vdst[:, :, 0:DH],
                        ps[:, :].rearrange("p (h c) -> p h c", c=DH),
                        bv_bc[:, :].rearrange("p (h c) -> p h c", c=DH))
                    nc.vector.memset(vdst[:, :, DH:DH + 1], 1.0)
                return [ua, ub]

            # ---- phase D: attention per (pair, q-range) ---------------------
            def attn_range(p, ri, hook=None):
                q0, qw = QRANGES[ri]
                kt = qk_sb[2 + p]
                qt_ = qk_sb[p]
                hA, hB = 2 * p, 2 * p + 1
                qs = slice(q0, q0 + qw)
                po0 = po_pool.tile([DH + 1, qw], F32, name="po0")
                po1 = po_pool.tile([DH + 1, qw], F32, name="po1")
                for c2 in range(KC // 2):
                    c, c1 = 2 * c2, 2 * c2 + 1
                    cs = slice(c * P, (c + 1) * P)
                    cs1 = slice(c1 * P, (c1 + 1) * P)
                    st, sp = (c == 0), (c1 == KC - 1)
                    swA = sw_pool.tile([P, 2 * qw], F32, name="sw")
                    nc.tensor.matmul(swA[:, 0:qw], kt[0:64, cs],
                                     qt_[0:64, qs], tile_position=(0, 0),
                                     start=True, stop=True)
                    nc.tensor.matmul(swA[:, qw:2 * qw], kt[0:64, cs1],
                                     qt_[0:64, qs], tile_position=(0, 0),
                                     start=True, stop=True)
                    ptA = pt_pool.tile([P, 2 * qw], BF16, name="pt")
                    nc.scalar.activation(
                        ptA[:], swA[:], mybir.ActivationFunctionType.Exp)
                    swB = sw_pool.tile([P, 2 * qw], F32, name="sw")
                    nc.tensor.matmul(swB[:, 0:qw], kt[64:128, cs],
                                     qt_[64:128, qs], tile_position=(64, 0),
                                     start=True, stop=True)
                    nc.tensor.matmul(swB[:, qw:2 * qw], kt[64:128, cs1],
                                     qt_[64:128, qs], tile_position=(64, 0),
                                     start=True, stop=True)
                    ptB = pt_pool.tile([P, 2 * qw], BF16, name="pt")
                    nc.scalar.activation(
                        ptB[:], swB[:], mybir.ActivationFunctionType.Exp)
                    nc.tensor.matmul(
                        po0[:], v_sb[c][:, hA * (DH + 1):(hA + 1) * (DH + 1)],
                        ptA[:, 0:qw], start=st, stop=False)
                    nc.tensor.matmul(
                        po0[:], v_sb[c1][:, hA * (DH + 1):(hA + 1) * (DH + 1)],
                        ptA[:, qw:2 * qw], start=False, stop=sp)
                    nc.tensor.matmul(
                        po1[:], v_sb[c][:, hB * (DH + 1):(hB + 1) * (DH + 1)],
                        ptB[:, 0:qw], start=st, stop=False)
                    nc.tensor.matmul(
                        po1[:], v_sb[c1][:, hB * (DH + 1):(hB + 1) * (DH + 1)],
                        ptB[:, qw:2 * qw], start=False, stop=sp)
                # normalize by 1/denominator (psum row 64, per q token)
                ot = o_pool.tile([P, qw], BF16, name="o")
                for hi, po in ((0, po0), (1, po1)):
                    d = nrm_pool.tile([1, qw], F32, name="d")
                    nc.vector.tensor_copy(d[0:1, :], po[64:65, :])
                    r = nrm_pool.tile([1, qw], F32, name="r")
                    scr = nrm_pool.tile([1, qw], F32, name="scr")
                    nc.vector.reciprocal_approx_accurate(
                        r[0:1, :], d[0:1, :], scr[0:1, :])
                    rb = nrm_pool.tile([64, qw], F32, name="rb")
                    nc.gpsimd.partition_broadcast(rb[0:64, :], r[0:1, :])
                    nc.vector.tensor_mul(
                        ot[64 * hi:64 * (hi + 1), :], po[0:64, :], rb[0:64, :])
                if ri == 5:
                    # pair-0 qt3 (full width) feeds both half-range buffers
                    nc.sync.dma_start(ag_in[3][0:P, :], ot[:, 0:NT // 2])
                    nc.sync.dma_start(ag_in[4][0:P, :], ot[:, NT // 2:NT])
                else:
                    nc.sync.dma_start(ag_in[ri][p * P:(p + 1) * P, :], ot[:])

            def ag_range(ri):
                nc.gpsimd.collective_compute(
                    "AllGather", mybir.AluOpType.bypass,
                    replica_groups=GROUPS,
                    ins=[ag_in[ri].ap().opt()],
                    outs=[ag_out[ri].ap().opt()])

            # ---- proj per q-range: wp stationary, out.T[of, q].  The final
            # out-DMAs are detached (emitted at the very end on sync) so they
            # never head-of-line-block later o->ag_in stores on that queue.
            pr_tiles = {}

            def proj_units(ri):
                q0, qw = QRANGES[ri]
                ofull = []

                def load():
                    # on gpsimd (SWDGE): its wait on the AllGather semaphore
                    # must not head-of-line-block the sync queue's o->ag_in
                    # stores; behind it on gpsimd are only later collective
                    # triggers, which are serial with this AG anyway.
                    for cg in range(8):
                        t = ofull_pool.tile([P, qw], BF16, name="ofull")
                        nc.gpsimd.dma_start(
                            t[:], ag_out[ri][cg * P:(cg + 1) * P, :])
                        ofull.append(t)
                units = [load]
                for h in range(2):
                    state = {}

                    def mk(h=h, half=0, state=state):
                        def u():
                            if half == 0:
                                state["ps"] = aux_pool.tile(
                                    [P, qw], F32, name="aux")
                            for cg in range(4 * half, 4 * half + 4):
                                g, pair = cg // 2, cg % 2
                                nc.tensor.matmul(
                                    state["ps"][:],
                                    wp_t[(pair, g)][:, h * P:(h + 1) * P],
                                    ofull[cg][:],
                                    start=(cg == 0), stop=(cg == 7))
                            if half == 1:
                                pr = pr_pool.tile([P, qw], F32,
                                                  name=f"pr{ri}{h}")
                                nc.vector.tensor_scalar_add(
                                    pr[:], state["ps"][:], bp_sb[:, h:h + 1])
                                pr_tiles[(ri, h)] = pr
                        return u
                    units.append(mk(h, 0, state))
                    units.append(mk(h, 1, state))
                return units

            def emit_out_stores():
                for (ri, h), pr in sorted(pr_tiles.items()):
                    q0, qw = QRANGES[ri]
                    nc.sync.dma_start(
                        out_ext[h * P:(h + 1) * P, q0:q0 + qw], pr[:])
                pr_tiles.clear()

            # ---- emission ---------------------------------------------------
            # (order also defines dependencies: producers before consumers;
            # v and pair-1 qk are priority-demoted gap fillers under phase D)
            qk_mtile(2)
            qk_mtile(0)
            with tc.high_priority(offset=-1_000_000):
                for t in range(KC):
                    for u in v_units(t):
                        u()
                qk_mtile(1)
                qk_mtile(3)
            attn_range(0, 0)
            attn_range(0, 1)
            attn_range(1, 0)
            ag_range(0)
            attn_range(0, 2)
            for u in proj_units(0):
                u()
            attn_range(1, 1)
            ag_range(1)
            attn_range(0, 5)
            for u in proj_units(1):
                u()
            attn_range(1, 2)
            ag_range(2)
            attn_range(1, 3)
            for u in proj_units(2):
                u()
            ag_range(3)
            attn_range(1, 4)
            ag_range(4)
            for u in proj_units(3):
                u()
            for u in proj_units(4):
                u()
            emit_out_stores()

    nc.compile()
    return nc


_NC_CACHE = None


def _get_nc():
    global _NC_CACHE
    if _NC_CACHE is None:
        _NC_CACHE = build_nc()
    return _NC_CACHE


def _bf16(a):
    return np.ascontiguousarray(a.astype(ml_dtypes.bfloat16))


def kernel(x, w_qkv, b_qkv, w_proj, b_proj):
    global LAST_RESULTS
    x = np.asarray(x, dtype=np.float32)
    w_qkv = np.asarray(w_qkv, dtype=np.float32)
    b_qkv = np.asarray(b_qkv, dtype=np.float32)
    w_proj = np.asarray(w_proj, dtype=np.float32)
    b_proj = np.asarray(b_proj, dtype=np.float32)

    nc = _get_nc()

    in_maps = []
    for c in CORE_IDS:
        b, g = c // 4, c % 4
        cs = slice(g * OF, (g + 1) * OF)   # feature cols of this head group
        wq = w_qkv[:, 0 * D:1 * D][:, cs] * SCALE
        wk = w_qkv[:, 1 * D:2 * D][:, cs]
        wv = w_qkv[:, 2 * D:3 * D][:, cs]
        bq = b_qkv[0 * D:1 * D][cs] * SCALE
        bk = b_qkv[1 * D:2 * D][cs]
        bqk = np.concatenate([bq, bk]).reshape(4, P).T.copy()  # [128, 4]
        in_maps.append({
            "xt": _bf16(x[b].T),
            "wqk": _bf16(np.concatenate([wq, wk], axis=1)),
            "wv": _bf16(wv),
            "bqk": np.ascontiguousarray(bqk, dtype=np.float32),
            "bv": np.ascontiguousarray(
                b_qkv[2 * D + g * OF:2 * D + (g + 1) * OF].reshape(1, OF)),
            "wp": _bf16(w_proj[:, cs]),
            "bp": np.ascontiguousarray(
                b_proj[cs].reshape(2, P).T, dtype=np.float32),
        })

    trace = bool(os.environ.get("KERNEL_TRACE"))
    if trace:
        _install_ntff_shim()
    LAST_RESULTS = run_bass_kernel_spmd(
        nc, in_maps, CORE_IDS, trace=trace)

    out = np.empty((B, N, D), dtype=np.float32)
    for c in CORE_IDS:
        b, g = c // 4, c % 4
        out[b, :, g * OF:(g + 1) * OF] = LAST_RESULTS.results[c]["out"].T
    return out



# revision 4
# speedup vs baseline: 1.0913x; 1.0913x over previous
"""Distributed multi-head attention kernel for one TRN2 chip (8 NeuronCores).

Problem: x[2,2048,1024] -> qkv -> 16-head attention -> out proj, f32 I/O.

Sharding: 8 cores = 2 batches x 4 head-groups (4 heads each).
Core c: batch b=c//4, head group g=c%4 (heads 4g..4g+3).

v2 structure (vs v1: 330us):
 - Inputs host-packed so every DMA moves >=2KB/partition-line, sliced by
   need-order: wqk m-slices (kt pair-0 first) and xt n-slices land first,
   so the first s-matmul fires ~4us in instead of ~47us.
 - Fill work (remaining qk m/n units, v production, proj) emitted in one
   priority-demoted block, hand-interleaved in deadline order; attention
   s->exp->PV chain at normal priority so the scheduler always prefers it.
 - Out-stores inline per range on the vector queue (it only carries the
   pr eviction the store depends on) instead of detached at the end.
 - pt pool deepened to 8 so the exp stream can run ~4 chunk-pairs ahead
   of PV when v production races just-ahead of consumption.

Device algorithm (per core), all matmuls bf16 with f32 PSUM accumulate:
 1) qkT = wqk.T @ xT   [512,2048]  (q weights/bias pre-scaled by
    1/sqrt(dh) on host; bias added during DVE eviction)
 2) v_aug = xT.T @ w_v [2048, 4*65] (bias via DVE; per-head ones column
    interleaved so the PV matmul also emits softmax denominators)
 3) per (pair, q-range): per chunk-pair: sT = k @ qT (row-tiled head
    pair), p = exp(sT) on ScalarE (PSUM->SBUF bf16), oT_aug += v_aug.T
    @ p (M=65: row 64 = denominator); normalize oT by 1/denom on DVE.
 4) per q-range: AllGather o of both pairs (256 rows); proj with w_proj
    chunks stationary: out.T[of, q] += wp.T @ o_full; + bias; DMA out
    [256, 2048] transposed (host un-transposes).
"""

import os
import sys
import types
import numpy as np
import ml_dtypes

import concourse.bass as bass
import concourse.mybir as mybir
import concourse.bacc as bacc
import concourse.tile as tile
from concourse.bass_utils import run_bass_kernel_spmd

BF16 = mybir.dt.bfloat16
F32 = mybir.dt.float32

B, N, D = 2, 2048, 1024
H, DH = 16, 64
SCALE = DH ** -0.5

P = 128                 # partitions
NT = 512                # token free-dim tile
KC = N // P             # 16 k-token chunks
QT = N // NT            # 4 q tiles
DC = D // P             # 8 d_model chunks
HPC = 4                 # heads per core
OF = HPC * DH           # 256 o-features per core
VW = HPC * (DH + 1)     # v_aug width (260): per head [v(64) | ones(1)]

CORE_IDS = list(range(8))
GROUPS = [[0, 1, 2, 3], [4, 5, 6, 7]]
LAST_RESULTS = None


def _install_ntff_shim():
    """Provide antenv.axon_hooks (absent from this image's antenv stub) so
    run_bass_kernel_spmd(trace=True) can reach the NTFF profiler in
    libaxon_pjrt.so. Only needed when profiling."""
    if "antenv.axon_hooks" in sys.modules:
        return
    try:
        from trn_agent_boot.trn_boot import _ntff_profile_via_ctypes
        hook = _ntff_profile_via_ctypes("/opt/axon/libaxon_pjrt.so")
    except Exception:
        hook = None
    mod = types.ModuleType("antenv.axon_hooks")
    mod._hook = hook
    mod.get_axon_ntff_profile_hook = lambda: mod._hook
    mod.set_axon_ntff_profile_hook = lambda h: setattr(mod, "_hook", h)
    sys.modules["antenv.axon_hooks"] = mod


def build_nc():
    nc = bacc.Bacc("TRN2", target_bir_lowering=False, debug=False, num_devices=8)

    # Host-packed inputs (k-chunk-major along free dim for wide DMA lines):
    #  xtp[n]  rows n*128..: [128, 8*512]  = xT[k*128:(k+1)*128, n*512:+512]
    #  wqkp[m] rows m*128..: [128, 8*128]  = wqk[k*128:(k+1)*128, m*128:+128]
    #  wvp: [128, 8*256], wpp: [128, 8*256]
    xtp_ext = nc.dram_tensor("xtp", [4 * P, DC * NT], BF16, kind="ExternalInput")
    wqkp_ext = nc.dram_tensor("wqkp", [4 * P, DC * P], BF16, kind="ExternalInput")
    wvp_ext = nc.dram_tensor("wvp", [P, DC * OF], BF16, kind="ExternalInput")
    wpp_ext = nc.dram_tensor("wpp", [P, DC * OF], BF16, kind="ExternalInput")
    bqk_ext = nc.dram_tensor("bqk", [P, 4], F32, kind="ExternalInput")
    bv_ext = nc.dram_tensor("bv", [1, OF], F32, kind="ExternalInput")
    bp_ext = nc.dram_tensor("bp", [P, 2], F32, kind="ExternalInput")
    # transposed output [of, q]; host transposes back
    out_ext = nc.dram_tensor("out", [OF, N], F32, kind="ExternalOutput")

    # AllGather bounce buffers per q-range (qt 0,1,2 full; qt3 in halves —
    # ranges 3,4 — so the tail gather pipelines; range 5 = full qt3, used
    # only by pair 0's attention which feeds both half buffers).
    QRANGES = [(0, NT), (NT, NT), (2 * NT, NT),
               (3 * NT, NT // 2), (3 * NT + NT // 2, NT // 2),
               (3 * NT, NT)]
    ag_in = [nc.dram_tensor(f"ag_in_{i}", [2 * P, qw], BF16)
             for i, (q0, qw) in enumerate(QRANGES[:5])]
    ag_out = [nc.dram_tensor(f"ag_out_{i}", [8 * P, qw], BF16)
              for i, (q0, qw) in enumerate(QRANGES[:5])]
    # dummy collective to absorb the bootstrap barrier at T=0
    agw_in = nc.dram_tensor("agw_in", [1, 16], BF16)
    agw_out = nc.dram_tensor("agw_out", [4, 16], BF16)

    with tile.TileContext(nc) as tc:
        with (
            tc.tile_pool(name="xt_pool", bufs=1) as xt_pool,
            tc.tile_pool(name="w_pool", bufs=1) as w_pool,
            tc.tile_pool(name="qk_pool", bufs=1) as qk_pool,
            tc.tile_pool(name="v_pool", bufs=1) as v_pool,
            tc.tile_pool(name="const_pool", bufs=1) as const_pool,
            tc.tile_pool(name="pt_pool", bufs=8) as pt_pool,
            tc.tile_pool(name="o_pool", bufs=4) as o_pool,
            tc.tile_pool(name="nrm_pool", bufs=2) as nrm_pool,
            tc.tile_pool(name="ofull_pool", bufs=10) as ofull_pool,
            tc.tile_pool(name="pr_pool", bufs=2) as pr_pool,
            tc.tile_pool(name="sw_pool", bufs=2, space="PSUM") as sw_pool,
            tc.tile_pool(name="po_pool", bufs=1, space="PSUM") as po_pool,
            tc.tile_pool(name="aux_pool", bufs=2, space="PSUM") as aux_pool,
        ):
            # ---- T=0 warmups ------------------------------------------------
            nc.gpsimd.collective_compute(
                "AllGather", mybir.AluOpType.bypass,
                replica_groups=GROUPS,
                ins=[agw_in.ap().opt()],
                outs=[agw_out.ap().opt()])
            warm_in = const_pool.tile([1, 16], F32)
            warm_out = const_pool.tile([1, 16], F32)
            nc.vector.memset(warm_in[:], 0.0)
            nc.scalar.activation(
                warm_out[:], warm_in[:], mybir.ActivationFunctionType.Exp)

            # ---- input loads, need-order, alternating the two HWDGE queues.
            # m order: kt pair0 (2), qt pair0 (0), kt pair1 (3), qt pair1 (1)
            qcount = [0]

            def q():
                qcount[0] += 1
                return nc.sync if qcount[0] % 2 else nc.scalar

            bqk_sb = const_pool.tile([P, 4], F32)
            q().dma_start(bqk_sb[:], bqk_ext[:])
            bv_row = const_pool.tile([1, OF], F32)
            q().dma_start(bv_row[:], bv_ext[:])

            wqk_m = {}
            for m in (2, 0):
                t = w_pool.tile([P, DC * P], BF16, name=f"wqk{m}")
                q().dma_start(t[:], wqkp_ext[m * P:(m + 1) * P, :])
                wqk_m[m] = t
            xt_n = []
            for n in range(4):
                t = xt_pool.tile([P, DC * NT], BF16, name=f"xt{n}")
                xt_n.append(t)
            q().dma_start(xt_n[0][:], xtp_ext[0:P, :])
            for m in (3, 1):
                t = w_pool.tile([P, DC * P], BF16, name=f"wqk{m}")
                q().dma_start(t[:], wqkp_ext[m * P:(m + 1) * P, :])
                wqk_m[m] = t
            wv_sb = w_pool.tile([P, DC * OF], BF16, name="wv")
            q().dma_start(wv_sb[:], wvp_ext[:])
            for n in range(1, 4):
                q().dma_start(xt_n[n][:], xtp_ext[n * P:(n + 1) * P, :])
            wp_sb = w_pool.tile([P, DC * OF], BF16, name="wp")
            q().dma_start(wp_sb[:], wpp_ext[:])
            bp_sb = const_pool.tile([P, 2], F32)
            q().dma_start(bp_sb[:], bp_ext[:])
            bv_bc = const_pool.tile([P, OF], F32)
            nc.gpsimd.partition_broadcast(bv_bc[:], bv_row[:])

            def xt_sl(k, n):          # [128, 512] token slice n of d-chunk k
                return xt_n[n][:, k * NT:(k + 1) * NT]

            def xt_ksl(k, t):         # [128, 128] token chunk t of d-chunk k
                n, r = divmod(t, 4)
                return xt_n[n][:, k * NT + r * P:k * NT + (r + 1) * P]

            def wqk_sl(m, k):         # [128, 128] d-chunk k of m-tile m
                return wqk_m[m][:, k * P:(k + 1) * P]

            def wv_sl(k):             # [128, 256]
                return wv_sb[:, k * OF:(k + 1) * OF]

            def wp_sl(cg, h):         # [128, 128] wp rows cg*128.., cols h*128..
                return wp_sb[:, cg * OF + h * P:cg * OF + (h + 1) * P]

            # ---- phase B: qkT = wqk.T @ xT -> 4 tiles [128, 2048] bf16 ------
            # m=0: q heads 0-1, m=1: q heads 2-3, m=2: k heads 0-1, m=3: k 2-3
            qk_sb = [qk_pool.tile([P, N], BF16, name=f"qk{m}") for m in range(4)]

            def qk_units(m, n):
                """qkT (m, n-slice) as 2 units of 4 k-chunks; DVE eviction
                with bias on the last."""
                state = {}

                def make(kh):
                    def u():
                        if kh == 0:
                            state["ps"] = aux_pool.tile([P, NT], F32, name="aux")
                        for k in range(4 * kh, 4 * kh + 4):
                            nc.tensor.matmul(
                                state["ps"][:], wqk_sl(m, k), xt_sl(k, n),
                                start=(k == 0), stop=(k == DC - 1))
                        if kh == 1:
                            nc.vector.tensor_scalar_add(
                                qk_sb[m][:, n * NT:(n + 1) * NT],
                                state["ps"][:], bqk_sb[:, m:m + 1])
                    return u
                return [make(0), make(1)]

            # ---- phase C: v_aug [2048, 260] bf16 (ones interleaved) ---------
            v_sb = [v_pool.tile([P, VW], BF16, name=f"v{t}") for t in range(KC)]

            def v_units(t):
                state = {}

                def ua():
                    state["ps"] = aux_pool.tile([P, OF], F32, name="aux")
                    for k in range(4):
                        nc.tensor.matmul(
                            state["ps"][:], xt_ksl(k, t), wv_sl(k),
                            start=(k == 0), stop=False)

                def ub():
                    ps = state["ps"]
                    for k in range(4, 8):
                        nc.tensor.matmul(
                            ps[:], xt_ksl(k, t), wv_sl(k),
                            start=False, stop=(k == DC - 1))
                    vdst = v_sb[t][:, :].rearrange("p (h c) -> p h c", c=DH + 1)
                    nc.vector.tensor_add(
                        vdst[:, :, 0:DH],
                        ps[:, :].rearrange("p (h c) -> p h c", c=DH),
                        bv_bc[:, :].rearrange("p (h c) -> p h c", c=DH))
                    nc.vector.memset(vdst[:, :, DH:DH + 1], 1.0)
                return [ua, ub]

            # ---- phase D: attention per (pair, q-range) ---------------------
            def attn_range(p, ri):
                q0, qw = QRANGES[ri]
                kt = qk_sb[2 + p]
                qt_ = qk_sb[p]
                hA, hB = 2 * p, 2 * p + 1
                qs = slice(q0, q0 + qw)
                po0 = po_pool.tile([DH + 1, qw], F32, name="po0")
                po1 = po_pool.tile([DH + 1, qw], F32, name="po1")
                for c2 in range(KC // 2):
                    c, c1 = 2 * c2, 2 * c2 + 1
                    cs = slice(c * P, (c + 1) * P)
                    cs1 = slice(c1 * P, (c1 + 1) * P)
                    st, sp = (c == 0), (c1 == KC - 1)
                    swA = sw_pool.tile([P, 2 * qw], F32, name="sw")
                    swB = sw_pool.tile([P, 2 * qw], F32, name="sw")
                    # interleave quadrants so adjacent MMs run concurrently
                    nc.tensor.matmul(swA[:, 0:qw], kt[0:64, cs],
                                     qt_[0:64, qs], tile_position=(0, 0),
                                     start=True, stop=True)
                    nc.tensor.matmul(swB[:, 0:qw], kt[64:128, cs],
                                     qt_[64:128, qs], tile_position=(64, 0),
                                     start=True, stop=True)
                    nc.tensor.matmul(swA[:, qw:2 * qw], kt[0:64, cs1],
                                     qt_[0:64, qs], tile_position=(0, 0),
                                     start=True, stop=True)
                    nc.tensor.matmul(swB[:, qw:2 * qw], kt[64:128, cs1],
                                     qt_[64:128, qs], tile_position=(64, 0),
                                     start=True, stop=True)
                    ptA = pt_pool.tile([P, 2 * qw], BF16, name="pt")
                    nc.scalar.activation(
                        ptA[:], swA[:], mybir.ActivationFunctionType.Exp)
                    ptB = pt_pool.tile([P, 2 * qw], BF16, name="pt")
                    nc.scalar.activation(
                        ptB[:], swB[:], mybir.ActivationFunctionType.Exp)
                    nc.tensor.matmul(
                        po0[:], v_sb[c][:, hA * (DH + 1):(hA + 1) * (DH + 1)],
                        ptA[:, 0:qw], start=st, stop=False)
                    nc.tensor.matmul(
                        po0[:], v_sb[c1][:, hA * (DH + 1):(hA + 1) * (DH + 1)],
                        ptA[:, qw:2 * qw], start=False, stop=sp)
                    nc.tensor.matmul(
                        po1[:], v_sb[c][:, hB * (DH + 1):(hB + 1) * (DH + 1)],
                        ptB[:, 0:qw], start=st, stop=False)
                    nc.tensor.matmul(
                        po1[:], v_sb[c1][:, hB * (DH + 1):(hB + 1) * (DH + 1)],
                        ptB[:, qw:2 * qw], start=False, stop=sp)
                # normalize by 1/denominator (psum row 64, per q token)
                ot = o_pool.tile([P, qw], BF16, name="o")
                for hi, po in ((0, po0), (1, po1)):
                    d = nrm_pool.tile([1, qw], F32, name="d")
                    nc.vector.tensor_copy(d[0:1, :], po[64:65, :])
                    r = nrm_pool.tile([1, qw], F32, name="r")
                    scr = nrm_pool.tile([1, qw], F32, name="scr")
                    nc.vector.reciprocal_approx_accurate(
                        r[0:1, :], d[0:1, :], scr[0:1, :])
                    rb = nrm_pool.tile([64, qw], F32, name="rb")
                    nc.gpsimd.partition_broadcast(rb[0:64, :], r[0:1, :])
                    nc.vector.tensor_mul(
                        ot[64 * hi:64 * (hi + 1), :], po[0:64, :], rb[0:64, :])
                if ri == 5:
                    # pair-0 qt3 (full width) feeds both half-range buffers
                    nc.sync.dma_start(ag_in[3][0:P, :], ot[:, 0:NT // 2])
                    nc.sync.dma_start(ag_in[4][0:P, :], ot[:, NT // 2:NT])
                else:
                    nc.sync.dma_start(ag_in[ri][p * P:(p + 1) * P, :], ot[:])

            def ag_range(ri):
                nc.gpsimd.collective_compute(
                    "AllGather", mybir.AluOpType.bypass,
                    replica_groups=GROUPS,
                    ins=[ag_in[ri].ap().opt()],
                    outs=[ag_out[ri].ap().opt()])

            # ---- proj per q-range: wp stationary, out.T[of, q].  Out-stores
            # inline on the vector HWDGE queue (it only carries the pr
            # eviction this store depends on).
            def proj_units(ri):
                q0, qw = QRANGES[ri]
                ofull = []

                def load():
                    # on gpsimd (SWDGE): its wait on the AllGather semaphore
                    # must not head-of-line-block the sync queue's o->ag_in
                    # stores; behind it on gpsimd are only later collective
                    # triggers, which are serial with this AG anyway.
                    for cg in range(8):
                        t = ofull_pool.tile([P, qw], BF16, name="ofull")
                        nc.gpsimd.dma_start(
                            t[:], ag_out[ri][cg * P:(cg + 1) * P, :])
                        ofull.append(t)
                units = [load]
                for h in range(2):
                    state = {}

                    def mk(h=h, half=0, state=state):
                        def u():
                            if half == 0:
                                state["ps"] = aux_pool.tile(
                                    [P, qw], F32, name="aux")
                            for cg in range(4 * half, 4 * half + 4):
                                g, pair = cg // 2, cg % 2
                                nc.tensor.matmul(
                                    state["ps"][:], wp_sl(cg, h), ofull[cg][:],
                                    start=(cg == 0), stop=(cg == 7))
                            if half == 1:
                                pr = pr_pool.tile([P, qw], F32, name="pr")
                                nc.vector.tensor_scalar_add(
                                    pr[:], state["ps"][:], bp_sb[:, h:h + 1])
                                nc.gpsimd.dma_start(
                                    out_ext[h * P:(h + 1) * P, q0:q0 + qw],
                                    pr[:])
                        return u
                    units.append(mk(h, 0, state))
                    units.append(mk(h, 1, state))
                return units

            # ---- emission ---------------------------------------------------
            # Producers before consumers (Tile deps follow emission order).
            # Critical chain at normal priority; everything that can yield is
            # in the demoted fill block, hand-interleaved in deadline order.
            for u in qk_units(2, 0):
                u()
            for u in qk_units(0, 0):
                u()
            with tc.high_priority(offset=-1_000_000):
                fill = []
                fill += qk_units(2, 1)          # kt p0 n1 (attn(0,0) c4-7)
                fill += v_units(0) + v_units(1)
                fill += qk_units(2, 2)
                fill += v_units(2) + v_units(3)
                fill += qk_units(2, 3)
                fill += v_units(4) + v_units(5)
                fill += qk_units(0, 1)          # qt p0 n1 (range (0,1))
                fill += v_units(6) + v_units(7)
                fill += qk_units(3, 0)          # kt p1 (range (1,0))
                fill += v_units(8) + v_units(9)
                fill += qk_units(3, 1)
                fill += v_units(10) + v_units(11)
                fill += qk_units(3, 2)
                fill += v_units(12) + v_units(13)
                fill += qk_units(3, 3)
                fill += v_units(14) + v_units(15)
                fill += qk_units(1, 0)          # qt p1 n0
                fill += qk_units(0, 2)          # qt p0 n2
                fill += qk_units(1, 1)
                fill += qk_units(0, 3)
                fill += qk_units(1, 2)
                fill += qk_units(1, 3)
                for u in fill:
                    u()
            attn_range(0, 0)
            attn_range(0, 1)
            attn_range(1, 0)
            ag_range(0)
            attn_range(1, 1)
            ag_range(1)
            with tc.high_priority(offset=-1_000_000):
                for u in proj_units(0):
                    u()
            attn_range(0, 2)
            with tc.high_priority(offset=-1_000_000):
                for u in proj_units(1):
                    u()
            attn_range(1, 2)
            ag_range(2)
            attn_range(0, 5)
            with tc.high_priority(offset=-1_000_000):
                for u in proj_units(2):
                    u()
            attn_range(1, 3)
            ag_range(3)
            attn_range(1, 4)
            ag_range(4)
            with tc.high_priority(offset=-1_000_000):
                for u in proj_units(3):
                    u()
                for u in proj_units(4):
                    u()

    nc.compile()
    return nc


_NC_CACHE = None


def _get_nc():
    global _NC_CACHE
    if _NC_CACHE is None:
        _NC_CACHE = build_nc()
    return _NC_CACHE


def _bf16(a):
    return np.ascontiguousarray(a.astype(ml_dtypes.bfloat16))


def kernel(x, w_qkv, b_qkv, w_proj, b_proj):
    global LAST_RESULTS
    x = np.asarray(x, dtype=np.float32)
    w_qkv = np.asarray(w_qkv, dtype=np.float32)
    b_qkv = np.asarray(b_qkv, dtype=np.float32)
    w_proj = np.asarray(w_proj, dtype=np.float32)
    b_proj = np.asarray(b_proj, dtype=np.float32)

    nc = _get_nc()

    in_maps = []
    for c in CORE_IDS:
        b, g = c // 4, c % 4
        cs = slice(g * OF, (g + 1) * OF)   # feature cols of this head group
        wq = w_qkv[:, 0 * D:1 * D][:, cs] * SCALE
        wk = w_qkv[:, 1 * D:2 * D][:, cs]
        wv = w_qkv[:, 2 * D:3 * D][:, cs]
        bq = b_qkv[0 * D:1 * D][cs] * SCALE
        bk = b_qkv[1 * D:2 * D][cs]
        bqk = np.concatenate([bq, bk]).reshape(4, P).T.copy()  # [128, 4]
        wqk = np.concatenate([wq, wk], axis=1)                 # [1024, 512]
        xt = x[b].T                                            # [1024, 2048]
        wp = w_proj[:, cs]                                     # [1024, 256]

        # packed layouts: k-chunk-major along the free dim
        xtp = np.empty((4 * P, DC * NT), np.float32)
        for n in range(4):
            for k in range(DC):
                xtp[n * P:(n + 1) * P, k * NT:(k + 1) * NT] = \
                    xt[k * P:(k + 1) * P, n * NT:(n + 1) * NT]
        wqkp = np.empty((4 * P, DC * P), np.float32)
        for m in range(4):
            for k in range(DC):
                wqkp[m * P:(m + 1) * P, k * P:(k + 1) * P] = \
                    wqk[k * P:(k + 1) * P, m * P:(m + 1) * P]
        wvp = np.empty((P, DC * OF), np.float32)
        wpp = np.empty((P, DC * OF), np.float32)
        for k in range(DC):
            wvp[:, k * OF:(k + 1) * OF] = wv[k * P:(k + 1) * P, :]
            wpp[:, k * OF:(k + 1) * OF] = wp[k * P:(k + 1) * P, :]

        in_maps.append({
            "xtp": _bf16(xtp),
            "wqkp": _bf16(wqkp),
            "wvp": _bf16(wvp),
            "wpp": _bf16(wpp),
            "bqk": np.ascontiguousarray(bqk, dtype=np.float32),
            "bv": np.ascontiguousarray(
                b_qkv[2 * D + g * OF:2 * D + (g + 1) * OF].reshape(1, OF)),
            "bp": np.ascontiguousarray(
                b_proj[cs].reshape(2, P).T, dtype=np.float32),
        })

    trace = bool(os.environ.get("KERNEL_TRACE"))
    if trace:
        _install_ntff_shim()
    LAST_RESULTS = run_bass_kernel_spmd(
        nc, in_maps, CORE_IDS, trace=trace)

    out = np.empty((B, N, D), dtype=np.float32)
    for c in CORE_IDS:
        b, g = c // 4, c % 4
        out[b, :, g * OF:(g + 1) * OF] = LAST_RESULTS.results[c]["out"].T
    return out
